# revision 16
# baseline (speedup 1.0000x reference)
"""Masked-LSTM Trainium2 kernel (v2).

Problem: T=2048, B=64, I=256, H=512 PyTorch-style LSTM where the hidden
state h is channel-masked (keep_mask) before feeding the next step.

Strategy:
  - Data parallel over batch: 8 cores x 8 batch each; weights replicated.
  - Only the keep_mask==1 channels participate in the coupled recurrence
    (masked channels have h == 0 forever).  The per-step weight sweep
    (the LDWEIGHTS-rate bound) covers just those nk channels: gates
    i/f/g/o restricted to kept rows, h-input (K) restricted to kept cols.
  - Masked channels' cell state c is still needed for the final c_n, but
    it feeds nothing until then, so it is computed per 64-step chunk as
    a batched matmul over the staged kept-h plus a DVE
    tensor_tensor_scan (c = sigmoid(f)*c + sigmoid(i)*tanh(g)).
  - Recurrence matmul: gates.T tiles [<=128, 8] = W_tile.T @ hT_chunk,
    weights stationary bf16, h moving bf16, fp32 PSUM.
  - x @ W_ih.T + b precompute runs chunk-by-chunk on the tensor engine;
    x_gates never touch DRAM.
  - Output written in device layout [128, T, tpg, 8] bf16; the host
    unpermutes channels and upcasts.
"""

import numpy as np
import ml_dtypes

BF16 = ml_dtypes.bfloat16
N_CORES = 8
TRACE = False          # set True to collect an NTFF profile + exec time
LAST_EXEC_NS = None    # populated after each run when TRACE
NEG = -1.0e4           # sigmoid(NEG) == 0: kills gates of padded lanes


def _build_and_run(x, keep_mask, W_ih, W_hh, b_ih, b_hh):
    import concourse.bass as bass
    import concourse.bacc as bacc
    import concourse.mybir as mybir
    from concourse.tile import TileContext
    from concourse.bass_utils import run_bass_kernel_spmd
    from concourse.bass import ds
    from contextlib import ExitStack

    T, B, I = x.shape
    H = W_hh.shape[1]
    BL = B // N_CORES
    assert B % N_CORES == 0 and I % 128 == 0 and H % 128 == 0

    keep = np.asarray(keep_mask).astype(bool)
    keep_idx = np.nonzero(keep)[0]
    drop_idx = np.nonzero(~keep)[0]
    nk, nd = len(keep_idx), len(drop_idx)
    assert 0 < nk and 0 < nd, "degenerate masks not supported"

    tpg = (nk + 127) // 128          # kept chunks (K side and per-gate tiles)
    S = nk - 128 * (tpg - 1)         # channels in the last kept chunk
    w = min(128, ((S + 31) // 32) * 32)  # padded width of the last chunk
    nf = tpg if w == 128 else tpg - 1    # full 128-wide tiles per gate
    npart = 0 if w == 128 else 4         # one partial tile per gate
    tpgd = (nd + 127) // 128             # dropped chunks
    kc_in = I // 128
    Mk = 4 * nf + npart                  # kept m-tiles
    Md = 3 * tpgd                        # dropped m-tiles (i,f,g)
    CH = 64
    UNROLL = 2
    assert T % (CH * UNROLL) == 0
    NIT = T // (CH * UNROLL)

    W_ih = np.asarray(W_ih); W_hh = np.asarray(W_hh)
    bsum = (np.asarray(b_ih) + np.asarray(b_hh)).astype(np.float32)

    # kept tiles, fulls first then partials: list of (gate, chunk, width)
    kept_tiles = [(g, c, 128) for g in range(4) for c in range(nf)]
    if npart:
        kept_tiles += [(g, tpg - 1, w) for g in range(4)]
    drop_tiles = [(g, d) for g in range(3) for d in range(tpgd)]

    def kept_rows(g, c, width):
        slots = np.arange(c * 128, c * 128 + width)
        valid = slots < nk
        rows = np.zeros(width, np.int64)
        rows[valid] = g * H + keep_idx[slots[valid]]
        return rows, valid

    def drop_rows(g, d):
        slots = np.arange(d * 128, d * 128 + 128)
        valid = slots < nd
        rows = np.zeros(128, np.int64)
        rows[valid] = g * H + drop_idx[slots[valid]]
        return rows, valid

    def kw(ck):  # K width of kept chunk ck
        return 128 if ck < tpg - 1 else w

    def kept_cols(ck):
        width = kw(ck)
        slots = np.arange(ck * 128, ck * 128 + width)
        valid = slots < nk
        cols = np.zeros(width, np.int64)
        cols[valid] = keep_idx[slots[valid]]
        return cols, valid

    # ---- pack all stationary weights into one bf16 buffer ----------------
    off = [0]
    offsets = {}

    def alloc(key, width):
        offsets[key] = off[0]
        off[0] += width

    for ck in range(tpg):               # recurrence tiles
        for ti in range(len(kept_tiles)):
            alloc(("rec", ck, ti), kept_tiles[ti][2])
    for kc in range(kc_in):             # kept x-gate tiles
        for ti in range(len(kept_tiles)):
            alloc(("prek", kc, ti), kept_tiles[ti][2])
    for ck in range(tpg):               # dropped-gate h tiles
        for ti in range(len(drop_tiles)):
            alloc(("ph2", ck, ti), 128)
    for kc in range(kc_in):             # dropped x-gate tiles
        for ti in range(len(drop_tiles)):
            alloc(("pred", kc, ti), 128)
    WCOLS = off[0]

    w_all = np.zeros((128, WCOLS), dtype=BF16)
    bias_np = np.zeros((128, Mk + Md), dtype=np.float32)

    for ti, (g, c, width) in enumerate(kept_tiles):
        rows, rvalid = kept_rows(g, c, width)
        bt = np.where(rvalid, bsum[rows], 0.0 if g == 2 else NEG)
        bias_np[:width, ti] = bt
        for ck in range(tpg):
            cols, cvalid = kept_cols(ck)
            tile = np.zeros((len(cols), width), np.float32)   # [K, M]
            tile[np.ix_(cvalid, rvalid)] = W_hh[np.ix_(rows[rvalid], cols[cvalid])].T
            o = offsets[("rec", ck, ti)]
            w_all[: len(cols), o:o + width] = tile.astype(BF16)
        for kc in range(kc_in):
            tile = np.zeros((128, width), np.float32)
            tile[:, rvalid] = W_ih[rows[rvalid], kc * 128:(kc + 1) * 128].T
            o = offsets[("prek", kc, ti)]
            w_all[:, o:o + width] = tile.astype(BF16)

    for ti, (g, d) in enumerate(drop_tiles):
        rows, rvalid = drop_rows(g, d)
        bt = np.where(rvalid, bsum[rows], 0.0 if g == 2 else NEG)
        bias_np[:, Mk + ti] = bt
        for ck in range(tpg):
            cols, cvalid = kept_cols(ck)
            tile = np.zeros((len(cols), 128), np.float32)
            tile[np.ix_(cvalid, rvalid)] = W_hh[np.ix_(rows[rvalid], cols[cvalid])].T
            o = offsets[("ph2", ck, ti)]
            w_all[: len(cols), o:o + 128] = tile.astype(BF16)
        for kc in range(kc_in):
            tile = np.zeros((128, 128), np.float32)
            tile[:, rvalid] = W_ih[rows[rvalid], kc * 128:(kc + 1) * 128].T
            o = offsets[("pred", kc, ti)]
            w_all[:, o:o + 128] = tile.astype(BF16)

    # xT per core: [128, kc_in, T*BL]
    x_np = np.asarray(x)
    xt_maps = []
    for j in range(N_CORES):
        xs = x_np[:, j * BL:(j + 1) * BL, :]
        xt = xs.transpose(2, 0, 1).reshape(I, T * BL)
        xt = xt.reshape(kc_in, 128, T * BL).transpose(1, 0, 2)
        xt_maps.append(np.ascontiguousarray(xt.astype(BF16)))

    # ---- build the bass program ------------------------------------------
    f32 = mybir.dt.float32
    bf16 = mybir.dt.bfloat16
    AF = mybir.ActivationFunctionType
    ALU = mybir.AluOpType

    nc = bacc.Bacc("TRN2", target_bir_lowering=False)
    xT_d = nc.declare_dram_parameter("xT", [128, kc_in, T * BL], bf16, isOutput=False)
    wall_d = nc.declare_dram_parameter("w_all", [128, WCOLS], bf16, isOutput=False)
    bias_d = nc.declare_dram_parameter("bias", [128, Mk + Md], f32, isOutput=False)
    out_d = nc.declare_dram_parameter("out_T", [128, T, tpg, BL], bf16, isOutput=True)
    coutk_d = nc.declare_dram_parameter("c_out_k", [128, tpg, BL], f32, isOutput=True)
    coutd_d = nc.declare_dram_parameter("c_out_d", [128, tpgd, BL], f32, isOutput=True)

    FB = nf * BL                 # cols per full gate region

    with ExitStack() as es:
        w_all_sb = es.enter_context(nc.sbuf_tensor("w_all_sb", [128, WCOLS], bf16))
        b_sb = es.enter_context(nc.sbuf_tensor("b_sb", [128, Mk + Md], f32))
        c_k = [es.enter_context(nc.sbuf_tensor(f"c_k{i}", [128, tpg * BL], f32))
               for i in range(2)]
        c_d = es.enter_context(nc.sbuf_tensor("c_d", [128, tpgd * BL], f32))
        h_carry = es.enter_context(nc.sbuf_tensor("h_carry", [128, tpg * BL], bf16))

        with TileContext(nc) as tc0:
            nc.sync.dma_start(w_all_sb[:], wall_d[:])
            nc.sync.dma_start(b_sb[:], bias_d[:])
            nc.gpsimd.memset(c_k[0][:], 0.0)
            nc.gpsimd.memset(c_d[:], 0.0)
            nc.gpsimd.memset(h_carry[:], 0.0)

        with TileContext(nc) as tc, ExitStack() as pools:
            xt_pool = pools.enter_context(tc.tile_pool(name="xt", bufs=2))
            xg_pool = pools.enter_context(tc.tile_pool(name="xg", bufs=2))
            xgd_pool = pools.enter_context(tc.tile_pool(name="xgd", bufs=2))
            stage_pool = pools.enter_context(tc.tile_pool(name="stage", bufs=2))
            work_pool = pools.enter_context(tc.tile_pool(name="work", bufs=3))
            ph2_pool = pools.enter_context(tc.tile_pool(name="ph2", bufs=2))
            scr_pool = pools.enter_context(tc.tile_pool(name="scr", bufs=2))
            big_ps_pool = pools.enter_context(
                tc.tile_pool(name="big_ps", bufs=2, space="PSUM"))
            rec_ps_pool = pools.enter_context(
                tc.tile_pool(name="rec_ps", bufs=2, space="PSUM"))

            def wsl(key, width, krows=128):
                o = offsets[key]
                return w_all_sb[:krows, o:o + width]

            def chunk_body(ci_expr, sub):
                xt_tile = xt_pool.tile([128, kc_in, CH * BL], bf16)
                nc.sync.dma_start(
                    xt_tile[:],
                    xT_d[:, :, ds(ci_expr * (UNROLL * CH * BL) + sub * CH * BL,
                                  CH * BL)],
                )
                stage = stage_pool.tile([128, CH + 1, tpg * BL], bf16)
                nc.vector.tensor_copy(stage[:, 0, :], h_carry[:])

                # --- x-gates for kept channels ---
                xg_tile = xg_pool.tile([128, CH, Mk * BL], f32)
                for ti, (g, c, width) in enumerate(kept_tiles):
                    ps = big_ps_pool.tile([128, CH * BL], f32, name="bps", uniquify=True)
                    for kc in range(kc_in):
                        nc.tensor.matmul(
                            ps[:width, :],
                            wsl(("prek", kc, ti), width),
                            xt_tile[:, kc, :],
                            start=(kc == 0), stop=(kc == kc_in - 1),
                        )
                    nc.scalar.activation(
                        xg_tile[:width, :, ti * BL:(ti + 1) * BL], ps[:width, :],
                        AF.Identity, bias=b_sb[:width, ti:ti + 1],
                    )
                # --- x-gates for dropped channels ---
                xgd_tile = xgd_pool.tile([128, CH, Md * BL], f32)
                for ti in range(Md):
                    ps = big_ps_pool.tile([128, CH * BL], f32, name="bps", uniquify=True)
                    for kc in range(kc_in):
                        nc.tensor.matmul(
                            ps[:],
                            wsl(("pred", kc, ti), 128),
                            xt_tile[:, kc, :],
                            start=(kc == 0), stop=(kc == kc_in - 1),
                        )
                    nc.scalar.activation(
                        xgd_tile[:, :, ti * BL:(ti + 1) * BL], ps[:],
                        AF.Identity, bias=b_sb[:, Mk + ti:Mk + ti + 1],
                    )

                # Fence: keep the chunk-setup work (pre-MMs and their big
                # ACT copies, previous chunk's phase-2) out of the per-step
                # dependency chain -- ACT/DVE are strict FIFO and a 686ns
                # foreign op inside the step chain stalls the whole step.
                tc.strict_bb_all_engine_barrier()

                # --- recurrence over kept channels ---
                for s in range(CH):
                    c_in = c_k[s % 2]
                    c_out = c_k[(s + 1) % 2]
                    ps = rec_ps_pool.tile([128, Mk * BL], f32)
                    for ti, (g, c, width) in enumerate(kept_tiles):
                        for ck in range(tpg):
                            nc.tensor.matmul(
                                ps[:width, ti * BL:(ti + 1) * BL],
                                wsl(("rec", ck, ti), width, kw(ck)),
                                stage[:kw(ck), s, ck * BL:(ck + 1) * BL],
                                start=(ck == 0), stop=(ck == tpg - 1),
                            )
                    # gates += x-gates; regions [i,f fulls][g fulls][o fulls][partials]
                    nc.vector.tensor_add(
                        ps[:, 0:2 * FB], ps[:, 0:2 * FB], xg_tile[:, s, 0:2 * FB])
                    nc.vector.tensor_add(
                        ps[:, 2 * FB:3 * FB], ps[:, 2 * FB:3 * FB],
                        xg_tile[:, s, 2 * FB:3 * FB])
                    nc.vector.tensor_add(
                        ps[:, 3 * FB:4 * FB], ps[:, 3 * FB:4 * FB],
                        xg_tile[:, s, 3 * FB:4 * FB])
                    sif = work_pool.tile([128, 2 * FB], f32, name="sif")
                    nc.scalar.activation(sif[:], ps[:, 0:2 * FB], AF.Sigmoid)
                    tgf = work_pool.tile([128, FB], f32, name="tgf")
                    nc.scalar.activation(tgf[:], ps[:, 2 * FB:3 * FB], AF.Tanh)
                    sof = work_pool.tile([128, FB], f32, name="sof")
                    nc.scalar.activation(sof[:], ps[:, 3 * FB:4 * FB], AF.Sigmoid)
                    v1 = work_pool.tile([128, FB], f32, name="v1")
                    nc.vector.tensor_mul(v1[:], sif[:, 0:FB], tgf[:])
                    v2 = work_pool.tile([128, FB], f32, name="v2")
                    nc.vector.tensor_mul(v2[:], sif[:, FB:2 * FB], c_in[:, 0:FB])
                    nc.vector.tensor_add(c_out[:, 0:FB], v1[:], v2[:])
                    tctf = work_pool.tile([128, FB], f32, name="tctf")
                    nc.scalar.activation(tctf[:], c_out[:, 0:FB], AF.Tanh)
                    nc.vector.tensor_mul(
                        stage[:, s + 1, 0:FB], sof[:], tctf[:])
                    if npart:
                        P0 = 4 * FB
                        nc.vector.tensor_add(
                            ps[:w, P0:P0 + 4 * BL], ps[:w, P0:P0 + 4 * BL],
                            xg_tile[:w, s, P0:P0 + 4 * BL])
                        sifp = work_pool.tile([w, 2 * BL], f32, name="sifp")
                        nc.scalar.activation(
                            sifp[:], ps[:w, P0:P0 + 2 * BL], AF.Sigmoid)
                        tgp = work_pool.tile([w, BL], f32, name="tgp")
                        nc.scalar.activation(
                            tgp[:], ps[:w, P0 + 2 * BL:P0 + 3 * BL], AF.Tanh)
                        sop = work_pool.tile([w, BL], f32, name="sop")
                        nc.scalar.activation(
                            sop[:], ps[:w, P0 + 3 * BL:P0 + 4 * BL], AF.Sigmoid)
                        v1p = work_pool.tile([w, BL], f32, name="v1p")
                        nc.vector.tensor_mul(v1p[:], sifp[:, 0:BL], tgp[:])
                        v2p = work_pool.tile([w, BL], f32, name="v2p")
                        nc.vector.tensor_mul(
                            v2p[:], sifp[:, BL:2 * BL], c_in[:w, FB:FB + BL])
                        nc.vector.tensor_add(c_out[:w, FB:FB + BL], v1p[:], v2p[:])
                        tctp = work_pool.tile([w, BL], f32, name="tctp")
                        nc.scalar.activation(
                            tctp[:], c_out[:w, FB:FB + BL], AF.Tanh)
                        nc.vector.tensor_mul(
                            stage[:w, s + 1, FB:FB + BL], sop[:], tctp[:])

                nc.vector.tensor_copy(h_carry[:], stage[:, CH, :])

                # --- phase 2: dropped-channel cell state over this chunk ---
                a_sb = ph2_pool.tile([128, tpgd, CH, BL], f32, name=f"a{sub}")
                si_sb = ph2_pool.tile([128, tpgd, CH, BL], f32, name=f"si{sub}")
                tg2_sb = ph2_pool.tile([128, tpgd, CH, BL], f32, name=f"tg2{sub}")
                for ti, (g, d) in enumerate(drop_tiles):
                    psd = big_ps_pool.tile([128, CH * BL], f32, name="bps", uniquify=True)
                    for ck in range(tpg):
                        nc.tensor.matmul(
                            psd[:],
                            wsl(("ph2", ck, ti), 128, kw(ck)),
                            stage[:kw(ck), 0:CH, ck * BL:(ck + 1) * BL],
                            start=(ck == 0), stop=(ck == tpg - 1),
                        )
                    nc.vector.tensor_add(
                        psd[:], psd[:], xgd_tile[:, :, ti * BL:(ti + 1) * BL])
                    dst = (si_sb, a_sb, tg2_sb)[g]
                    fn = (AF.Sigmoid, AF.Sigmoid, AF.Tanh)[g]
                    nc.scalar.activation(dst[:, d, :, :], psd[:], fn)
                b_sb2 = ph2_pool.tile([128, tpgd, CH, BL], f32, name=f"b2{sub}")
                nc.vector.tensor_mul(b_sb2[:], si_sb[:], tg2_sb[:])
                for d in range(tpgd):
                    for b in range(BL):
                        scr = scr_pool.tile([128, CH], f32, name="scr")
                        nc.vector.tensor_tensor_scan(
                            scr[:],
                            a_sb[:, d, :, b],
                            b_sb2[:, d, :, b],
                            c_d[:, d * BL + b:d * BL + b + 1],
                            op0=ALU.mult, op1=ALU.add,
                        )
                        nc.vector.tensor_copy(
                            c_d[:, d * BL + b:d * BL + b + 1], scr[:, CH - 1:CH])

                nc.sync.dma_start(
                    out_d[:, ds(ci_expr * (UNROLL * CH) + sub * CH, CH), :, :],
                    stage[:, 1:CH + 1, :],
                )

            with tc.For_i(0, NIT, 1) as ci:
                for sub in range(UNROLL):
                    chunk_body(ci, sub)

            nc.sync.dma_start(coutk_d[:], c_k[0][:])
            nc.sync.dma_start(coutd_d[:], c_d[:])

    nc.compile()

    in_maps = [
        {"xT": xt_maps[j], "w_all": w_all, "bias": bias_np}
        for j in range(N_CORES)
    ]
    global LAST_EXEC_NS
    res = run_bass_kernel_spmd(nc, in_maps, list(range(N_CORES)), trace=TRACE)
    LAST_EXEC_NS = res.exec_time_ns

    # ---- host-side reassembly --------------------------------------------
    out_full = np.zeros((T, B, H), dtype=np.float32)
    c_full = np.zeros((1, B, H), dtype=np.float32)
    for j in range(N_CORES):
        r = res.results[j]
        ot = np.asarray(r["out_T"], dtype=np.float32)       # [128, T, tpg, BL]
        hperm = ot.transpose(1, 3, 2, 0).reshape(T, BL, tpg * 128)
        out_full[:, j * BL:(j + 1) * BL, :][..., keep_idx] = hperm[..., :nk]
        ck = np.asarray(r["c_out_k"], dtype=np.float32)     # [128, tpg, BL]
        ckp = ck.transpose(2, 1, 0).reshape(BL, tpg * 128)
        c_full[0, j * BL:(j + 1) * BL][:, keep_idx] = ckp[:, :nk]
        cd = np.asarray(r["c_out_d"], dtype=np.float32)
        cdp = cd.transpose(2, 1, 0).reshape(BL, tpgd * 128)
        c_full[0, j * BL:(j + 1) * BL][:, drop_idx] = cdp[:, :nd]
    h_full = out_full[-1][None].copy()
    return out_full, (h_full, c_full)


def kernel(x, keep_mask, W_ih, W_hh, b_ih, b_hh):
    out, (h_n, c_n) = _build_and_run(
        np.asarray(x, dtype=np.float32),
        np.asarray(keep_mask),
        np.asarray(W_ih, dtype=np.float32),
        np.asarray(W_hh, dtype=np.float32),
        np.asarray(b_ih, dtype=np.float32),
        np.asarray(b_hh, dtype=np.float32),
    )
    return out, (h_n, c_n)


# revision 17
# speedup vs baseline: 1.0641x; 1.0641x over previous
"""Masked-LSTM Trainium2 kernel (v2).

Problem: T=2048, B=64, I=256, H=512 PyTorch-style LSTM where the hidden
state h is channel-masked (keep_mask) before feeding the next step.

Strategy:
  - Data parallel over batch: 8 cores x 8 batch each; weights replicated.
  - Only the keep_mask==1 channels participate in the coupled recurrence
    (masked channels have h == 0 forever).  The per-step weight sweep
    (the LDWEIGHTS-rate bound) covers just those nk channels: gates
    i/f/g/o restricted to kept rows, h-input (K) restricted to kept cols.
  - Masked channels' cell state c is still needed for the final c_n, but
    it feeds nothing until then, so it is computed per 64-step chunk as
    a batched matmul over the staged kept-h plus a DVE
    tensor_tensor_scan (c = sigmoid(f)*c + sigmoid(i)*tanh(g)).
  - Recurrence matmul: gates.T tiles [<=128, 8] = W_tile.T @ hT_chunk,
    weights stationary bf16, h moving bf16, fp32 PSUM.
  - x @ W_ih.T + b precompute runs chunk-by-chunk on the tensor engine;
    x_gates never touch DRAM.
  - Output written in device layout [128, T, tpg, 8] bf16; the host
    unpermutes channels and upcasts.
"""

import numpy as np
import ml_dtypes

BF16 = ml_dtypes.bfloat16
N_CORES = 8
TRACE = False          # set True to collect an NTFF profile + exec time
LAST_EXEC_NS = None    # populated after each run when TRACE
NEG = -1.0e4           # sigmoid(NEG) == 0: kills gates of padded lanes


def _build_and_run(x, keep_mask, W_ih, W_hh, b_ih, b_hh):
    import concourse.bass as bass
    import concourse.bacc as bacc
    import concourse.mybir as mybir
    from concourse.tile import TileContext
    from concourse.bass_utils import run_bass_kernel_spmd
    from concourse.bass import ds
    from contextlib import ExitStack

    T, B, I = x.shape
    H = W_hh.shape[1]
    BL = B // N_CORES
    assert B % N_CORES == 0 and I % 128 == 0 and H % 128 == 0

    keep = np.asarray(keep_mask).astype(bool)
    keep_idx = np.nonzero(keep)[0]
    drop_idx = np.nonzero(~keep)[0]
    nk, nd = len(keep_idx), len(drop_idx)
    assert 0 < nk and 0 < nd, "degenerate masks not supported"

    tpg = (nk + 127) // 128          # kept chunks (K side and per-gate tiles)
    S = nk - 128 * (tpg - 1)         # channels in the last kept chunk
    w = min(128, ((S + 31) // 32) * 32)  # padded width of the last chunk
    nf = tpg if w == 128 else tpg - 1    # full 128-wide tiles per gate
    npart = 0 if w == 128 else 4         # one partial tile per gate
    tpgd = (nd + 127) // 128             # dropped chunks
    kc_in = I // 128
    Mk = 4 * nf + npart                  # kept m-tiles
    Md = 3 * tpgd                        # dropped m-tiles (i,f,g)
    CH = 64
    UNROLL = 2
    assert T % (CH * UNROLL) == 0
    NIT = T // (CH * UNROLL)

    W_ih = np.asarray(W_ih); W_hh = np.asarray(W_hh)
    bsum = (np.asarray(b_ih) + np.asarray(b_hh)).astype(np.float32)

    # kept tiles, fulls first then partials: list of (gate, chunk, width)
    kept_tiles = [(g, c, 128) for g in range(4) for c in range(nf)]
    if npart:
        kept_tiles += [(g, tpg - 1, w) for g in range(4)]
    drop_tiles = [(g, d) for g in range(3) for d in range(tpgd)]

    def kept_rows(g, c, width):
        slots = np.arange(c * 128, c * 128 + width)
        valid = slots < nk
        rows = np.zeros(width, np.int64)
        rows[valid] = g * H + keep_idx[slots[valid]]
        return rows, valid

    def drop_rows(g, d):
        slots = np.arange(d * 128, d * 128 + 128)
        valid = slots < nd
        rows = np.zeros(128, np.int64)
        rows[valid] = g * H + drop_idx[slots[valid]]
        return rows, valid

    def kw(ck):  # K width of kept chunk ck
        return 128 if ck < tpg - 1 else w

    def kept_cols(ck):
        width = kw(ck)
        slots = np.arange(ck * 128, ck * 128 + width)
        valid = slots < nk
        cols = np.zeros(width, np.int64)
        cols[valid] = keep_idx[slots[valid]]
        return cols, valid

    # ---- pack all stationary weights into one bf16 buffer ----------------
    off = [0]
    offsets = {}

    def alloc(key, width):
        offsets[key] = off[0]
        off[0] += width

    for ck in range(tpg):               # recurrence tiles
        for ti in range(len(kept_tiles)):
            alloc(("rec", ck, ti), kept_tiles[ti][2])
    for kc in range(kc_in):             # kept x-gate tiles
        for ti in range(len(kept_tiles)):
            alloc(("prek", kc, ti), kept_tiles[ti][2])
    for ck in range(tpg):               # dropped-gate h tiles
        for ti in range(len(drop_tiles)):
            alloc(("ph2", ck, ti), 128)
    for kc in range(kc_in):             # dropped x-gate tiles
        for ti in range(len(drop_tiles)):
            alloc(("pred", kc, ti), 128)
    WCOLS = off[0]

    w_all = np.zeros((128, WCOLS), dtype=BF16)
    bias_np = np.zeros((128, Mk + Md), dtype=np.float32)

    for ti, (g, c, width) in enumerate(kept_tiles):
        rows, rvalid = kept_rows(g, c, width)
        bt = np.where(rvalid, bsum[rows], 0.0 if g == 2 else NEG)
        bias_np[:width, ti] = bt
        for ck in range(tpg):
            cols, cvalid = kept_cols(ck)
            tile = np.zeros((len(cols), width), np.float32)   # [K, M]
            tile[np.ix_(cvalid, rvalid)] = W_hh[np.ix_(rows[rvalid], cols[cvalid])].T
            o = offsets[("rec", ck, ti)]
            w_all[: len(cols), o:o + width] = tile.astype(BF16)
        for kc in range(kc_in):
            tile = np.zeros((128, width), np.float32)
            tile[:, rvalid] = W_ih[rows[rvalid], kc * 128:(kc + 1) * 128].T
            o = offsets[("prek", kc, ti)]
            w_all[:, o:o + width] = tile.astype(BF16)

    for ti, (g, d) in enumerate(drop_tiles):
        rows, rvalid = drop_rows(g, d)
        bt = np.where(rvalid, bsum[rows], 0.0 if g == 2 else NEG)
        bias_np[:, Mk + ti] = bt
        for ck in range(tpg):
            cols, cvalid = kept_cols(ck)
            tile = np.zeros((len(cols), 128), np.float32)
            tile[np.ix_(cvalid, rvalid)] = W_hh[np.ix_(rows[rvalid], cols[cvalid])].T
            o = offsets[("ph2", ck, ti)]
            w_all[: len(cols), o:o + 128] = tile.astype(BF16)
        for kc in range(kc_in):
            tile = np.zeros((128, 128), np.float32)
            tile[:, rvalid] = W_ih[rows[rvalid], kc * 128:(kc + 1) * 128].T
            o = offsets[("pred", kc, ti)]
            w_all[:, o:o + 128] = tile.astype(BF16)

    # xT per core: [128, kc_in, T*BL]
    x_np = np.asarray(x)
    xt_maps = []
    for j in range(N_CORES):
        xs = x_np[:, j * BL:(j + 1) * BL, :]
        xt = xs.transpose(2, 0, 1).reshape(I, T * BL)
        xt = xt.reshape(kc_in, 128, T * BL).transpose(1, 0, 2)
        xt_maps.append(np.ascontiguousarray(xt.astype(BF16)))

    # ---- build the bass program ------------------------------------------
    f32 = mybir.dt.float32
    bf16 = mybir.dt.bfloat16
    AF = mybir.ActivationFunctionType
    ALU = mybir.AluOpType

    nc = bacc.Bacc("TRN2", target_bir_lowering=False)
    xT_d = nc.declare_dram_parameter("xT", [128, kc_in, T * BL], bf16, isOutput=False)
    wall_d = nc.declare_dram_parameter("w_all", [128, WCOLS], bf16, isOutput=False)
    bias_d = nc.declare_dram_parameter("bias", [128, Mk + Md], f32, isOutput=False)
    out_d = nc.declare_dram_parameter("out_T", [128, T, tpg, BL], bf16, isOutput=True)
    coutk_d = nc.declare_dram_parameter("c_out_k", [128, tpg, BL], f32, isOutput=True)
    coutd_d = nc.declare_dram_parameter("c_out_d", [128, tpgd, BL], f32, isOutput=True)

    FB = nf * BL                 # cols per full gate region

    with ExitStack() as es:
        w_all_sb = es.enter_context(nc.sbuf_tensor("w_all_sb", [128, WCOLS], bf16))
        b_sb = es.enter_context(nc.sbuf_tensor("b_sb", [128, Mk + Md], f32))
        c_k = [es.enter_context(nc.sbuf_tensor(f"c_k{i}", [128, tpg * BL], f32))
               for i in range(2)]
        c_d = es.enter_context(nc.sbuf_tensor("c_d", [128, tpgd * BL], f32))
        h_carry = es.enter_context(nc.sbuf_tensor("h_carry", [128, tpg * BL], bf16))

        with TileContext(nc) as tc0:
            nc.sync.dma_start(w_all_sb[:], wall_d[:])
            nc.sync.dma_start(b_sb[:], bias_d[:])
            nc.gpsimd.memset(c_k[0][:], 0.0)
            nc.gpsimd.memset(c_d[:], 0.0)
            nc.gpsimd.memset(h_carry[:], 0.0)

        with TileContext(nc) as tc, ExitStack() as pools:
            xt_pool = pools.enter_context(tc.tile_pool(name="xt", bufs=2))
            xg_pool = pools.enter_context(tc.tile_pool(name="xg", bufs=2))
            xgd_pool = pools.enter_context(tc.tile_pool(name="xgd", bufs=2))
            stage_pool = pools.enter_context(tc.tile_pool(name="stage", bufs=2))
            work_pool = pools.enter_context(tc.tile_pool(name="work", bufs=3))
            ph2_pool = pools.enter_context(tc.tile_pool(name="ph2", bufs=2))
            scr_pool = pools.enter_context(tc.tile_pool(name="scr", bufs=2))
            big_ps_pool = pools.enter_context(
                tc.tile_pool(name="big_ps", bufs=2, space="PSUM"))
            rec_ps_pool = pools.enter_context(
                tc.tile_pool(name="rec_ps", bufs=4, space="PSUM"))

            def wsl(key, width, krows=128):
                o = offsets[key]
                return w_all_sb[:krows, o:o + width]

            def chunk_body(ci_expr, sub):
                xt_tile = xt_pool.tile([128, kc_in, CH * BL], bf16)
                nc.sync.dma_start(
                    xt_tile[:],
                    xT_d[:, :, ds(ci_expr * (UNROLL * CH * BL) + sub * CH * BL,
                                  CH * BL)],
                )
                stage = stage_pool.tile([128, CH + 1, tpg * BL], bf16)
                nc.vector.tensor_copy(stage[:, 0, :], h_carry[:])

                # --- x-gates for kept channels ---
                xg_tile = xg_pool.tile([128, CH, Mk * BL], f32)
                for ti, (g, c, width) in enumerate(kept_tiles):
                    ps = big_ps_pool.tile([128, CH * BL], f32, name="bps", uniquify=True)
                    for kc in range(kc_in):
                        nc.tensor.matmul(
                            ps[:width, :],
                            wsl(("prek", kc, ti), width),
                            xt_tile[:, kc, :],
                            start=(kc == 0), stop=(kc == kc_in - 1),
                        )
                    nc.scalar.activation(
                        xg_tile[:width, :, ti * BL:(ti + 1) * BL], ps[:width, :],
                        AF.Identity, bias=b_sb[:width, ti:ti + 1],
                    )
                # --- x-gates for dropped channels ---
                xgd_tile = xgd_pool.tile([128, CH, Md * BL], f32)
                for ti in range(Md):
                    ps = big_ps_pool.tile([128, CH * BL], f32, name="bps", uniquify=True)
                    for kc in range(kc_in):
                        nc.tensor.matmul(
                            ps[:],
                            wsl(("pred", kc, ti), 128),
                            xt_tile[:, kc, :],
                            start=(kc == 0), stop=(kc == kc_in - 1),
                        )
                    nc.scalar.activation(
                        xgd_tile[:, :, ti * BL:(ti + 1) * BL], ps[:],
                        AF.Identity, bias=b_sb[:, Mk + ti:Mk + ti + 1],
                    )

                # Fence: keep the chunk-setup work (pre-MMs and their big
                # ACT copies, previous chunk's phase-2) out of the per-step
                # dependency chain -- ACT/DVE are strict FIFO and a 686ns
                # foreign op inside the step chain stalls the whole step.
                tc.strict_bb_all_engine_barrier()

                # --- recurrence over kept channels ---
                for s in range(CH):
                    c_in = c_k[s % 2]
                    c_out = c_k[(s + 1) % 2]
                    ps = rec_ps_pool.tile([128, Mk * BL], f32)
                    for ti, (g, c, width) in enumerate(kept_tiles):
                        for ck in range(tpg):
                            nc.tensor.matmul(
                                ps[:width, ti * BL:(ti + 1) * BL],
                                wsl(("rec", ck, ti), width, kw(ck)),
                                stage[:kw(ck), s, ck * BL:(ck + 1) * BL],
                                start=(ck == 0), stop=(ck == tpg - 1),
                            )
                    # gates += x-gates (one add over all full regions)
                    nc.vector.tensor_add(
                        ps[:, 0:4 * FB], ps[:, 0:4 * FB], xg_tile[:, s, 0:4 * FB])
                    if npart:
                        P0 = 4 * FB
                        nc.vector.tensor_add(
                            ps[:w, P0:P0 + 4 * BL], ps[:w, P0:P0 + 4 * BL],
                            xg_tile[:w, s, P0:P0 + 4 * BL])
                    sif = work_pool.tile([128, 2 * FB], f32, name="sif")
                    nc.scalar.activation(sif[:], ps[:, 0:2 * FB], AF.Sigmoid)
                    tgf = work_pool.tile([128, FB], f32, name="tgf")
                    nc.scalar.activation(tgf[:], ps[:, 2 * FB:3 * FB], AF.Tanh)
                    sof = work_pool.tile([128, FB], f32, name="sof")
                    nc.scalar.activation(sof[:], ps[:, 3 * FB:4 * FB], AF.Sigmoid)
                    if npart:
                        sifp = work_pool.tile([w, 2 * BL], f32, name="sifp")
                        nc.scalar.activation(
                            sifp[:], ps[:w, P0:P0 + 2 * BL], AF.Sigmoid)
                        tgp = work_pool.tile([w, BL], f32, name="tgp")
                        nc.scalar.activation(
                            tgp[:], ps[:w, P0 + 2 * BL:P0 + 3 * BL], AF.Tanh)
                        sop = work_pool.tile([w, BL], f32, name="sop")
                        nc.scalar.activation(
                            sop[:], ps[:w, P0 + 3 * BL:P0 + 4 * BL], AF.Sigmoid)
                    v1 = work_pool.tile([128, FB], f32, name="v1")
                    nc.vector.tensor_mul(v1[:], sif[:, 0:FB], tgf[:])
                    v2 = work_pool.tile([128, FB], f32, name="v2")
                    nc.vector.tensor_mul(v2[:], sif[:, FB:2 * FB], c_in[:, 0:FB])
                    nc.vector.tensor_add(c_out[:, 0:FB], v1[:], v2[:])
                    tctf = work_pool.tile([128, FB], f32, name="tctf")
                    nc.scalar.activation(tctf[:], c_out[:, 0:FB], AF.Tanh)
                    nc.vector.tensor_mul(
                        stage[:, s + 1, 0:FB], sof[:], tctf[:])
                    if npart:
                        v1p = work_pool.tile([w, BL], f32, name="v1p")
                        nc.vector.tensor_mul(v1p[:], sifp[:, 0:BL], tgp[:])
                        v2p = work_pool.tile([w, BL], f32, name="v2p")
                        nc.vector.tensor_mul(
                            v2p[:], sifp[:, BL:2 * BL], c_in[:w, FB:FB + BL])
                        nc.vector.tensor_add(c_out[:w, FB:FB + BL], v1p[:], v2p[:])
                        tctp = work_pool.tile([w, BL], f32, name="tctp")
                        nc.scalar.activation(
                            tctp[:], c_out[:w, FB:FB + BL], AF.Tanh)
                        nc.vector.tensor_mul(
                            stage[:w, s + 1, FB:FB + BL], sop[:], tctp[:])

                nc.vector.tensor_copy(h_carry[:], stage[:, CH, :])

                # --- phase 2: dropped-channel cell state over this chunk ---
                a_sb = ph2_pool.tile([128, tpgd, CH, BL], f32, name=f"a{sub}")
                si_sb = ph2_pool.tile([128, tpgd, CH, BL], f32, name=f"si{sub}")
                tg2_sb = ph2_pool.tile([128, tpgd, CH, BL], f32, name=f"tg2{sub}")
                for ti, (g, d) in enumerate(drop_tiles):
                    psd = big_ps_pool.tile([128, CH * BL], f32, name="bps", uniquify=True)
                    for ck in range(tpg):
                        nc.tensor.matmul(
                            psd[:],
                            wsl(("ph2", ck, ti), 128, kw(ck)),
                            stage[:kw(ck), 0:CH, ck * BL:(ck + 1) * BL],
                            start=(ck == 0), stop=(ck == tpg - 1),
                        )
                    nc.vector.tensor_add(
                        psd[:], psd[:], xgd_tile[:, :, ti * BL:(ti + 1) * BL])
                    dst = (si_sb, a_sb, tg2_sb)[g]
                    fn = (AF.Sigmoid, AF.Sigmoid, AF.Tanh)[g]
                    nc.scalar.activation(dst[:, d, :, :], psd[:], fn)
                b_sb2 = ph2_pool.tile([128, tpgd, CH, BL], f32, name=f"b2{sub}")
                nc.vector.tensor_mul(b_sb2[:], si_sb[:], tg2_sb[:])
                for d in range(tpgd):
                    for b in range(BL):
                        scr = scr_pool.tile([128, CH], f32, name="scr")
                        nc.vector.tensor_tensor_scan(
                            scr[:],
                            a_sb[:, d, :, b],
                            b_sb2[:, d, :, b],
                            c_d[:, d * BL + b:d * BL + b + 1],
                            op0=ALU.mult, op1=ALU.add,
                        )
                        nc.vector.tensor_copy(
                            c_d[:, d * BL + b:d * BL + b + 1], scr[:, CH - 1:CH])

                nc.sync.dma_start(
                    out_d[:, ds(ci_expr * (UNROLL * CH) + sub * CH, CH), :, :],
                    stage[:, 1:CH + 1, :],
                )

            with tc.For_i(0, NIT, 1) as ci:
                for sub in range(UNROLL):
                    chunk_body(ci, sub)

            nc.sync.dma_start(coutk_d[:], c_k[0][:])
            nc.sync.dma_start(coutd_d[:], c_d[:])

    nc.compile()

    in_maps = [
        {"xT": xt_maps[j], "w_all": w_all, "bias": bias_np}
        for j in range(N_CORES)
    ]
    global LAST_EXEC_NS
    res = run_bass_kernel_spmd(nc, in_maps, list(range(N_CORES)), trace=TRACE)
    LAST_EXEC_NS = res.exec_time_ns

    # ---- host-side reassembly --------------------------------------------
    out_full = np.zeros((T, B, H), dtype=np.float32)
    c_full = np.zeros((1, B, H), dtype=np.float32)
    for j in range(N_CORES):
        r = res.results[j]
        ot = np.asarray(r["out_T"], dtype=np.float32)       # [128, T, tpg, BL]
        hperm = ot.transpose(1, 3, 2, 0).reshape(T, BL, tpg * 128)
        out_full[:, j * BL:(j + 1) * BL, :][..., keep_idx] = hperm[..., :nk]
        ck = np.asarray(r["c_out_k"], dtype=np.float32)     # [128, tpg, BL]
        ckp = ck.transpose(2, 1, 0).reshape(BL, tpg * 128)
        c_full[0, j * BL:(j + 1) * BL][:, keep_idx] = ckp[:, :nk]
        cd = np.asarray(r["c_out_d"], dtype=np.float32)
        cdp = cd.transpose(2, 1, 0).reshape(BL, tpgd * 128)
        c_full[0, j * BL:(j + 1) * BL][:, drop_idx] = cdp[:, :nd]
    h_full = out_full[-1][None].copy()
    return out_full, (h_full, c_full)


def kernel(x, keep_mask, W_ih, W_hh, b_ih, b_hh):
    out, (h_n, c_n) = _build_and_run(
        np.asarray(x, dtype=np.float32),
        np.asarray(keep_mask),
        np.asarray(W_ih, dtype=np.float32),
        np.asarray(W_hh, dtype=np.float32),
        np.asarray(b_ih, dtype=np.float32),
        np.asarray(b_hh, dtype=np.float32),
    )
    return out, (h_n, c_n)
